# revision 1
# baseline (speedup 1.0000x reference)
"""Expert-parallel Conv1dBlock (Conv1d + GroupNorm + Mish) for Trainium2.

Strategy: 8 experts -> 8 NeuronCores. The host routes each sample to its
expert's core (MoE dispatch done as the sharding step), pads every core to a
common sample count, and each core runs an identical Bass/Tile program:

  - conv1d as matmuls over (Cin x K) contraction, fp32 precision recovered
    from fast float32r (11-bit mantissa) matmuls via a hi/lo split:
       W @ x ~= Wh@xh + Wh@xl + Wl@xh   (lo*lo term ~2^-24, dropped)
  - GroupNorm stats: per-partition sums ride free on ACT/DVE accumulators,
    cross-partition group reduction via tiny 0/1 matmuls
  - rsqrt(var+eps) via fast-inverse-sqrt bit trick + 3 Newton steps on DVE
    (keeps the ACT table set fixed: Identity/Square/Mish live in one set)
  - normalize + per-channel affine + Mish fused into a single ACT pass
    (out = Mish(y*scale + shift) with per-partition scale/bias APs)
"""

import sys

if "/opt/trn_rl_repo" not in sys.path:
    sys.path.insert(0, "/opt/trn_rl_repo")

import numpy as np

B, C, T = 512, 256, 256
E, KS, G = 8, 5, 8
EPS = 1e-5
HALF = C // 2  # 128, channels per partition block
GRP = C // G  # 32 channels per group
TP = T + 4  # padded time axis (2 halo columns each side)

PW = 8          # pairs per wave (stats batch)
NTERMS = 3      # 3 = fp32-accurate hi/lo split, 1 = single f32r pass
TRACE = False   # set True (module-global) to run with NTFF profiling
DO_STATS = True    # debug: False -> conv+bias only
DO_MISH = True     # debug: False -> affine only (no exp/square/recip)
LAST_EXEC_NS = None
LAST_RESULTS = None

_prog_cache = {}


def _round_f32r(x):
    """Round fp32 to the FP32R grid (1+8+11 bits, RNE) — matches walrus
    fp32_to_fp32r (downconv to 8-exp/11-mantissa)."""
    u = np.ascontiguousarray(x, dtype=np.float32).view(np.uint32)
    r = u + (0x7FF + ((u >> 12) & 1))
    r &= 0xFFFFF000
    out = r.view(np.float32)
    return out


def _install_trace_hook():
    import types

    if "antenv.axon_hooks" not in sys.modules:
        mod = types.ModuleType("antenv.axon_hooks")
        holder = [None]
        mod.set_axon_ntff_profile_hook = lambda h: holder.__setitem__(0, h)
        mod.get_axon_ntff_profile_hook = lambda: holder[0]
        sys.modules["antenv.axon_hooks"] = mod
        import antenv

        antenv.axon_hooks = mod
        from trn_agent_boot.trn_boot import _ntff_profile_via_ctypes

        mod.set_axon_ntff_profile_hook(
            _ntff_profile_via_ctypes("/opt/axon/libaxon_pjrt.so")
        )
    from concourse import bass_utils

    bass_utils.upload_artifacts = lambda tmpdir: f"local:{tmpdir}"


def _build_program(NP):
    import concourse.bacc as bacc
    import concourse.tile as tile
    from concourse import mybir

    dt = mybir.dt
    alu = mybir.AluOpType
    act = mybir.ActivationFunctionType

    nc = bacc.Bacc(None, target_bir_lowering=False)

    xh = nc.dram_tensor("xh", [NP, 2, HALF, 2 * TP], dt.float32r, kind="ExternalInput")
    xl = nc.dram_tensor("xl", [NP, 2, HALF, 2 * TP], dt.float32r, kind="ExternalInput")
    # weights laid out [co_blk, ci_blk, ci, k, co]
    wh = nc.dram_tensor("wh", [2, 2, HALF, KS, HALF], dt.float32r, kind="ExternalInput")
    wl = nc.dram_tensor("wl", [2, 2, HALF, KS, HALF], dt.float32r, kind="ExternalInput")
    bias2 = nc.dram_tensor("bias2", [HALF, 2], dt.float32, kind="ExternalInput")
    gamma2 = nc.dram_tensor("gamma2", [HALF, 2], dt.float32, kind="ExternalInput")
    beta2 = nc.dram_tensor("beta2", [HALF, 2], dt.float32, kind="ExternalInput")
    gmat = nc.dram_tensor("gmat", [2, HALF, HALF], dt.float32r, kind="ExternalInput")
    amat = nc.dram_tensor("amat", [2, HALF, HALF], dt.float32r, kind="ExternalInput")
    yo = nc.dram_tensor("yo", [NP, 2, HALF, 2, T], dt.float32, kind="ExternalOutput")

    n_waves = (NP + PW - 1) // PW
    inv_n = 1.0 / (GRP * T)

    with tile.TileContext(nc) as tc:
        import contextlib

        with contextlib.ExitStack() as ctx:
            singles = ctx.enter_context(tc.tile_pool(name="singles", bufs=1))
            xpool = ctx.enter_context(tc.tile_pool(name="xpool", bufs=3))
            cpsum = ctx.enter_context(tc.tile_pool(name="cpsum", bufs=2, space="PSUM"))
            ybpool = ctx.enter_context(tc.tile_pool(name="ybpool", bufs=2 * PW + 2))
            y2pool = ctx.enter_context(tc.tile_pool(name="y2pool", bufs=2))
            swpool = ctx.enter_context(tc.tile_pool(name="swpool", bufs=2))
            spsum = ctx.enter_context(tc.tile_pool(name="spsum", bufs=2, space="PSUM"))
            bpsum = ctx.enter_context(tc.tile_pool(name="bpsum", bufs=1, space="PSUM"))
            statp = ctx.enter_context(tc.tile_pool(name="statp", bufs=2))
            stp = ctx.enter_context(tc.tile_pool(name="stp", bufs=2))
            otpool = ctx.enter_context(tc.tile_pool(name="otpool", bufs=4))

            # ---- constants / weights resident in SBUF ----
            wsb_h = singles.tile([HALF, 2, 2, KS, HALF], dt.float32r)
            wsb_l = singles.tile([HALF, 2, 2, KS, HALF], dt.float32r)
            for cb in range(2):
                for cib in range(2):
                    nc.sync.dma_start(out=wsb_h[:, cb, cib, :, :], in_=wh[cb, cib])
                    if NTERMS == 3:
                        nc.sync.dma_start(out=wsb_l[:, cb, cib, :, :], in_=wl[cb, cib])
            bias_s = singles.tile([HALF, 2], dt.float32)
            nc.sync.dma_start(out=bias_s, in_=bias2[:, :])
            gamma_s = singles.tile([HALF, 2], dt.float32)
            nc.sync.dma_start(out=gamma_s, in_=gamma2[:, :])
            beta_s = singles.tile([HALF, 2], dt.float32)
            nc.sync.dma_start(out=beta_s, in_=beta2[:, :])
            gmat_s = singles.tile([HALF, 2, HALF], dt.float32r)
            nc.sync.dma_start(out=gmat_s, in_=gmat.rearrange("c p g -> p c g"))
            amat_s = singles.tile([HALF, 2, HALF], dt.float32r)
            nc.sync.dma_start(out=amat_s, in_=amat.rearrange("c g p -> g c p"))
            magic_s = singles.tile([G, 2 * PW], dt.int32)
            nc.vector.memset(magic_s, 0x5F3759DF)

            ztpool = ctx.enter_context(tc.tile_pool(name="ztpool", bufs=2))
            wpool = ctx.enter_context(tc.tile_pool(name="wpool", bufs=2))
            rpool = ctx.enter_context(tc.tile_pool(name="rpool", bufs=2))

            # state carried between waves for the deferred Mish+store pass
            prev_wave = None  # (list of (p, yb0, yb1, iw_base), scol, tcol)

            def emit_mish(wave_state):
                # mish(z) = z * tanh(softplus(z)) = z * (1 - 2/((1+e^z)^2+1))
                # keeps ACT inside the exp_and_others table set (no switches).
                pairs, scols, tcols = wave_state
                for (p, ybs, iw0) in pairs:
                    for cb in range(2):
                        zt = ztpool.tile([HALF, 2, T], dt.float32, name=f"zt{cb}",
                                         tag=f"zt{cb}")
                        if not DO_STATS:
                            nc.vector.tensor_copy(zt, ybs[cb])
                        else:
                            for s in range(2):
                                iw = iw0 + s
                                nc.vector.tensor_scalar(
                                    out=zt[:, s, :], in0=ybs[cb][:, s, :],
                                    scalar1=scols[cb][:, iw:iw + 1],
                                    scalar2=tcols[cb][:, iw:iw + 1],
                                    op0=alu.mult, op1=alu.add)
                        if not DO_MISH:
                            nc.sync.dma_start(out=yo[p, cb], in_=zt)
                            continue
                        w = wpool.tile([HALF, 2, T], dt.float32, name=f"w{cb}",
                                       tag=f"w{cb}")
                        nc.scalar.activation(out=w, in_=zt, func=act.Exp)
                        # v2 = (w+1)^2, then a = v2+1, in place
                        nc.scalar.activation(out=w, in_=w, func=act.Square, bias=1.0)
                        nc.vector.tensor_scalar(out=w, in0=w, scalar1=1.0,
                                                scalar2=None, op0=alu.add)
                        rsc = rpool.tile([HALF, 2, T], dt.float32, name="rsc",
                                         tag="rsc")
                        rr = rpool.tile([HALF, 2, T], dt.float32, name="rr", tag="rr")
                        nc.vector.reciprocal_approx_accurate(out=rr, in_=w,
                                                             scratch=rsc)
                        nc.vector.tensor_scalar(out=rr, in0=rr, scalar1=-2.0,
                                                scalar2=1.0, op0=alu.mult,
                                                op1=alu.add)
                        ot = otpool.tile([HALF, 2, T], dt.float32, name=f"ot{cb}",
                                         tag=f"ot{cb}")
                        nc.vector.tensor_tensor(out=ot, in0=zt, in1=rr, op=alu.mult)
                        nc.sync.dma_start(out=yo[p, cb], in_=ot)

            for w in range(n_waves):
                p0 = w * PW
                p1 = min(p0 + PW, NP)
                nw2 = 2 * (p1 - p0)
                sw = [swpool.tile([HALF, 2 * PW, 2], dt.float32, name=f"sw{cb}",
                                  tag=f"sw{cb}") for cb in range(2)]
                if nw2 < 2 * PW:
                    nc.vector.memset(sw[0], 0.0)
                    nc.vector.memset(sw[1], 0.0)
                wave_pairs = []
                for p in range(p0, p1):
                    iw0 = 2 * (p - p0)
                    xt_h = []
                    xt_l = []
                    for cib in range(2):
                        th = xpool.tile([HALF, 2, TP], dt.float32r,
                                        name=f"xh{cib}", tag=f"xh{cib}")
                        nc.sync.dma_start(out=th, in_=xh[p, cib].rearrange(
                            "p (s t) -> p s t", s=2))
                        xt_h.append(th)
                        if NTERMS == 3:
                            tl = xpool.tile([HALF, 2, TP], dt.float32r,
                                            name=f"xl{cib}", tag=f"xl{cib}")
                            nc.sync.dma_start(out=tl, in_=xl[p, cib].rearrange(
                                "p (s t) -> p s t", s=2))
                            xt_l.append(tl)
                    ybs = []
                    for cb in range(2):
                        cp = cpsum.tile([HALF, 2, T], dt.float32, name=f"cp{cb}",
                                        tag=f"cp{cb}")
                        # one accumulation group covering both samples in the
                        # bank: only the very first matmul carries start=True
                        # (it clears has_written for the whole bank; later
                        # first-touches overwrite, repeats accumulate).
                        first = True
                        for s in range(2):
                            for cib in range(2):
                                for k in range(KS):
                                    rh = xt_h[cib][:, s, k:k + T]
                                    terms = [(wsb_h[:, cb, cib, k, :], rh)]
                                    if NTERMS == 3:
                                        terms.append((wsb_h[:, cb, cib, k, :],
                                                      xt_l[cib][:, s, k:k + T]))
                                        terms.append((wsb_l[:, cb, cib, k, :], rh))
                                    group_last = (s == 1 and cib == 1
                                                  and k == KS - 1)
                                    for ti, (wi, ri) in enumerate(terms):
                                        nc.tensor.matmul(
                                            cp[:, s, :], wi, ri, start=first,
                                            stop=(group_last
                                                  and ti == len(terms) - 1))
                                        first = False
                        yb = ybpool.tile([HALF, 2, T], dt.float32, name=f"yb{cb}",
                                         tag=f"yb{cb}")
                        for s in range(2):
                            nc.scalar.activation(
                                out=yb[:, s, :], in_=cp[:, s, :], func=act.Identity,
                                bias=bias_s[:, cb:cb + 1],
                                accum_out=(sw[cb][:, iw0 + s, 0:1]
                                           if DO_STATS else None))
                            if DO_STATS:
                                y2 = y2pool.tile([HALF, T], dt.float32, name="y2",
                                                 tag="y2")
                                # sumsq of (y+bias): Square reads PSUM directly
                                # with the bias folded into the ACT affine.
                                nc.scalar.activation(
                                    out=y2, in_=cp[:, s, :], func=act.Square,
                                    bias=bias_s[:, cb:cb + 1],
                                    accum_out=sw[cb][:, iw0 + s, 1:2])
                        ybs.append(yb)
                    wave_pairs.append((p, ybs, iw0))

                # ---- wave statistics ----
                if not DO_STATS:
                    if prev_wave is not None:
                        emit_mish(prev_wave)
                    prev_wave = (wave_pairs, None, None)
                    continue
                # matmul moving operands must be single-free-dim: feed the
                # whole sw tile flattened (partial waves were memset to 0).
                sp = spsum.tile([HALF, 2 * PW * 2], dt.float32, name="sp",
                                tag="sp")
                for cb in range(2):
                    swf = sw[cb].rearrange("p a b -> p (a b)")
                    swh = statp.tile([HALF, 2 * PW * 2], dt.float32r,
                                     name=f"swh{cb}", tag=f"swh{cb}")
                    nc.vector.tensor_copy(swh, swf)
                    swl = statp.tile([HALF, 2 * PW * 2], dt.float32r,
                                     name=f"swl{cb}", tag=f"swl{cb}")
                    nc.vector.tensor_tensor(out=swl, in0=swf,
                                            in1=swh.bitcast(dt.float32),
                                            op=alu.subtract)
                    nc.tensor.matmul(sp, gmat_s[:, cb, :], swh,
                                     start=(cb == 0), stop=False)
                    nc.tensor.matmul(sp, gmat_s[:, cb, :], swl,
                                     start=False, stop=(cb == 1))
                spv = sp.rearrange("p (a b) -> p a b", b=2)

                R = statp.tile([HALF, 2, 2 * PW], dt.float32, name="R", tag="R")
                nc.vector.memset(R, 0.0)
                negmu = R[0:G, 0, :nw2]
                nc.vector.tensor_scalar(out=negmu, in0=spv[0:G, :nw2, 0],
                                        scalar1=-inv_n, scalar2=None, op0=alu.mult)
                m2e = statp.tile([G, 2 * PW], dt.float32, name="m2e", tag="m2e")
                nc.vector.tensor_scalar(out=m2e[:, :nw2], in0=spv[0:G, :nw2, 1],
                                        scalar1=inv_n, scalar2=EPS,
                                        op0=alu.mult, op1=alu.add)
                ve = statp.tile([G, 2 * PW], dt.float32, name="ve", tag="ve")
                nc.vector.tensor_tensor(out=ve[:, :nw2], in0=negmu, in1=negmu,
                                        op=alu.mult)
                nc.vector.tensor_tensor(out=ve[:, :nw2], in0=m2e[:, :nw2],
                                        in1=ve[:, :nw2], op=alu.subtract)
                # rsqrt via bit trick + Newton (all on DVE, tiny tiles)
                yi = statp.tile([G, 2 * PW], dt.int32, name="yi", tag="yi")
                nc.vector.tensor_scalar(out=yi[:, :nw2],
                                        in0=ve[:, :nw2].bitcast(dt.int32),
                                        scalar1=1, scalar2=None,
                                        op0=alu.arith_shift_right)
                nc.vector.tensor_tensor(out=yi[:, :nw2], in0=magic_s[:, :nw2],
                                        in1=yi[:, :nw2], op=alu.subtract)
                yf = yi.bitcast(dt.float32)
                xh2 = statp.tile([G, 2 * PW], dt.float32, name="xh2", tag="xh2")
                nc.vector.tensor_scalar(out=xh2[:, :nw2], in0=ve[:, :nw2],
                                        scalar1=0.5, scalar2=None, op0=alu.mult)
                aa = statp.tile([G, 2 * PW], dt.float32, name="aa", tag="aa")
                dd = statp.tile([G, 2 * PW], dt.float32, name="dd", tag="dd")
                for it in range(3):
                    nc.vector.tensor_tensor(out=aa[:, :nw2], in0=yf[:, :nw2],
                                            in1=yf[:, :nw2], op=alu.mult)
                    nc.vector.tensor_tensor(out=aa[:, :nw2], in0=xh2[:, :nw2],
                                            in1=aa[:, :nw2], op=alu.mult)
                    nc.vector.tensor_scalar(out=dd[:, :nw2], in0=aa[:, :nw2],
                                            scalar1=-1.0, scalar2=1.5,
                                            op0=alu.mult, op1=alu.add)
                    outp = R[0:G, 1, :nw2] if it == 2 else yf[:, :nw2]
                    nc.vector.tensor_tensor(out=outp, in0=yf[:, :nw2],
                                            in1=dd[:, :nw2], op=alu.mult)

                Rf = R.rearrange("p a b -> p (a b)")
                Rh = statp.tile([HALF, 2 * 2 * PW], dt.float32r, name="Rh", tag="Rh")
                nc.vector.tensor_copy(Rh, Rf)
                Rl = statp.tile([HALF, 2 * 2 * PW], dt.float32r, name="Rl", tag="Rl")
                nc.vector.tensor_tensor(out=Rl, in0=Rf,
                                        in1=Rh.bitcast(dt.float32),
                                        op=alu.subtract)
                scols = []
                tcols = []
                for cb in range(2):
                    bpf = bpsum.tile([HALF, 2 * 2 * PW], dt.float32, name=f"bp{cb}",
                                     tag=f"bp{cb}")
                    nc.tensor.matmul(bpf, amat_s[:, cb, :], Rh,
                                     start=True, stop=False)
                    nc.tensor.matmul(bpf, amat_s[:, cb, :], Rl,
                                     start=False, stop=True)
                    bp = bpf.rearrange("p (a b) -> p a b", a=2)
                    scol = stp.tile([HALF, 2 * PW], dt.float32, name=f"scol{cb}",
                                    tag=f"scol{cb}")
                    nc.vector.tensor_scalar(out=scol[:, :nw2], in0=bp[:, 1, :nw2],
                                            scalar1=gamma_s[:, cb:cb + 1],
                                            scalar2=None, op0=alu.mult)
                    tcol = stp.tile([HALF, 2 * PW], dt.float32, name=f"tcol{cb}",
                                    tag=f"tcol{cb}")
                    nc.vector.tensor_tensor(out=tcol[:, :nw2], in0=bp[:, 0, :nw2],
                                            in1=scol[:, :nw2], op=alu.mult)
                    nc.vector.tensor_scalar(out=tcol[:, :nw2], in0=tcol[:, :nw2],
                                            scalar1=beta_s[:, cb:cb + 1],
                                            scalar2=None, op0=alu.add)
                    scols.append(scol)
                    tcols.append(tcol)

                if prev_wave is not None:
                    emit_mish(prev_wave)
                prev_wave = (wave_pairs, scols, tcols)

            emit_mish(prev_wave)

    nc.finalize()
    return nc


def kernel(x, use_expert_i, W, b, gamma, beta):
    global LAST_EXEC_NS, LAST_RESULTS
    from concourse.bass_utils import run_bass_kernel_spmd

    if TRACE:
        _install_trace_hook()

    x = np.asarray(x, dtype=np.float32)
    u = np.asarray(use_expert_i).astype(np.int64)
    W = np.asarray(W, dtype=np.float32)
    b = np.asarray(b, dtype=np.float32)
    gamma = np.asarray(gamma, dtype=np.float32)
    beta = np.asarray(beta, dtype=np.float32)

    counts = np.bincount(u, minlength=E)
    n_max = max(int(counts.max()), 2)
    NP = (n_max + 1) // 2

    key = NP
    if key not in _prog_cache:
        _prog_cache[key] = _build_program(NP)
    nc = _prog_cache[key]

    # ---- host-side dispatch (the sharding step) ----
    idx_lists = []
    in_maps = []
    # group-indicator matrices, shared across cores
    gmat = np.zeros((2, HALF, HALF), np.float32)
    amat = np.zeros((2, HALF, HALF), np.float32)
    for cb in range(2):
        for p in range(HALF):
            g = cb * (G // 2) + p // GRP
            gmat[cb, p, g] = 1.0
            amat[cb, g, p] = 1.0

    for e in range(E):
        idx = np.nonzero(u == e)[0]
        pad_to = NP * 2
        if len(idx) == 0:
            padded = np.zeros(pad_to, np.int64)
        else:
            padded = np.concatenate([idx, np.full(pad_to - len(idx), idx[0])])
        idx_lists.append((idx, padded))

        xs = x[padded]  # [2*NP, C, T]
        # padded layout [NP, ci_blk, 128, 2*(T+4)] with zero halo columns
        xpad = np.zeros((NP, 2, HALF, 2, TP), np.float32)
        xv = xs.reshape(NP, 2, 2, HALF, T).transpose(0, 2, 3, 1, 4)
        xpad[:, :, :, :, 2:2 + T] = xv
        xph = _round_f32r(xpad)
        xpl = _round_f32r(xpad - xph)

        # weights [co_blk, ci_blk, ci, k, co]
        we = W[e].reshape(2, HALF, 2, HALF, KS).transpose(0, 2, 3, 4, 1)
        we = np.ascontiguousarray(we)
        weh = _round_f32r(we)
        wel = _round_f32r(we - weh)

        in_maps.append({
            "xh": xph.reshape(NP, 2, HALF, 2 * TP),
            "xl": xpl.reshape(NP, 2, HALF, 2 * TP),
            "wh": weh,
            "wl": wel,
            "bias2": np.ascontiguousarray(b[e].reshape(2, HALF).T),
            "gamma2": np.ascontiguousarray(gamma[e].reshape(2, HALF).T),
            "beta2": np.ascontiguousarray(beta[e].reshape(2, HALF).T),
            "gmat": gmat,
            "amat": amat,
        })

    res = run_bass_kernel_spmd(nc, in_maps, list(range(E)), trace=TRACE)
    LAST_EXEC_NS = res.exec_time_ns
    LAST_RESULTS = res

    out = np.empty((B, C, T), np.float32)
    for e in range(E):
        idx, padded = idx_lists[e]
        yo = res.results[e]["yo"]  # [NP, 2, 128, 2, T]
        ye = yo.transpose(0, 3, 1, 2, 4).reshape(NP * 2, C, T)
        if len(idx):
            out[idx] = ye[: len(idx)]
    return out

